# revision 7
# baseline (speedup 1.0000x reference)
"""NeighborSample Trainium2 kernel — disjoint-partition row-pair layout (v9).

Input  x:   (8, 64, 64, 192) f32
Output:     (8*64*64, 5, 5, 192) f32 — out[b*4096 + h*64 + w, i, j, c] =
            x[b, h+i-2, w+j-2, c] (zero-padded).

SDMA engine model distilled from v1/v3/v5/v7 traces:
  - 3840 B descriptors stream at 149 ns (25.8 GB/s per engine) when fed
    back-to-back; >4 KB packets drop to ~14 GB/s (no intra-packet pipelining);
  - two HWDGE rings interleave cleanly on one engine at full speed (v1
    engine 70: alternating q1/q10, all packets 149 ns);
  - concurrent reads of the SAME SBUF partition by different engines
    serialize (v5 put left/right halves on the same partitions via two
    tensors: uniform 280 ns packets).

v9 therefore:
  - host-pads x to xp [68, 68, 192];
  - bufP [68, 13824]: partition r holds TWO padded rows (2r, 2r+1) of one
    w-half: left half rows on partitions 0..33, right half on 34..67 —
    the two queues' working sets are partition-DISJOINT;
  - stores: 40 uniform DMAs (half s x i x h-segment g x h-parity phi), each
    [[2*OUT_H,16],[OUT_W,32],[1,960]] vs src [[ROW2,16],[C,32],[1,960]] —
    512 descriptors of 3840 B, outer count 16 = all-16-engine fan-out; the
    h-parity phi makes the padded-row index 2u+phi+i resolve to an affine
    partition index u + (phi+i-psi)//2, row-within-partition psi;
  - q1 (sync ring): left loads + left stores; q10 (scalar): right; no
    cross-queue waits at all; g=0 stores start once the 32-partition main
    load of their half lands.
"""

import sys

for _p in ("/opt/trn_rl_repo",):
    if _p not in sys.path:
        sys.path.insert(0, _p)

import numpy as np

import concourse.bass as bass
import concourse.mybir as mybir
from concourse.bass_utils import run_bass_kernel_spmd

B = 8
H = W = 64
C = 192
K = 5
PAD = 2
HALF = 32                # w positions per half
COLS = 36                # cols per half-row in bufP
HROW = COLS * C          # 6912 elems per padded half-row
ROW2 = 2 * HROW          # 13824 elems per bufP partition (two rows)
NPART = 34               # partitions per half (68 rows / 2)
PROWS = H + 2 * PAD      # 68 padded rows
PXROW = PROWS * C        # 13056 elems per padded-input row
WIN = K * C              # 960 (3840 B descriptor)
OUT_W = K * K * C        # 4800
OUT_H = W * OUT_W        # 307200
USEG = 16                # u values (h pairs) per store DMA


def build_nc() -> bass.Bass:
    nc = bass.Bass()
    xp = nc.declare_dram_parameter(
        "xp", [PROWS, PROWS, C], mybir.dt.float32, isOutput=False
    )
    out = nc.declare_dram_parameter(
        "out", [H, W, K, K, C], mybir.dt.float32, isOutput=True
    )

    with (
        nc.Block() as block,
        nc.semaphore("lmL") as lmL,
        nc.semaphore("ltL") as ltL,
        nc.semaphore("lmR") as lmR,
        nc.semaphore("ltR") as ltR,
        nc.semaphore("sS") as sS,
        nc.semaphore("sA") as sA,
        nc.sbuf_tensor("bufP", [128, ROW2], mybir.dt.float32) as bufP,
    ):

        def emit_half(eng, s, lm, lt, my_sem):
            p0 = s * NPART      # first partition of this half
            col0 = s * HALF * C
            # main load: partitions p0..p0+31 <- xp rows 0..63 of this half
            eng.dma_start(
                out=bass.AP(bufP, p0 * ROW2, [[ROW2, 32], [HROW, 2], [1, HROW]]),
                in_=bass.AP(xp, col0, [[2 * PXROW, 32], [PXROW, 2], [1, HROW]]),
            ).then_inc(lm, 16)
            # tail load: partitions p0+32..p0+33 <- xp rows 64..67
            eng.dma_start(
                out=bass.AP(
                    bufP, (p0 + 32) * ROW2, [[ROW2, 2], [HROW, 2], [1, HROW]]
                ),
                in_=bass.AP(
                    xp, 64 * PXROW + col0, [[2 * PXROW, 2], [PXROW, 2], [1, HROW]]
                ),
            ).then_inc(lt, 16)

            eng.wait_ge(lm, 16)
            njobs = 0
            for g in range(2):
                if g == 1:
                    eng.wait_ge(lt, 16)
                for i in range(K):
                    for phi in range(2):
                        psi = (phi + i) & 1
                        rbase = p0 + 16 * g + (phi + i - psi) // 2
                        eng.dma_start(
                            out=bass.AP(
                                out,
                                (32 * g + phi) * OUT_H
                                + s * HALF * OUT_W
                                + i * WIN,
                                [[2 * OUT_H, USEG], [OUT_W, HALF], [1, WIN]],
                            ),
                            in_=bass.AP(
                                bufP,
                                rbase * ROW2 + psi * HROW,
                                [[ROW2, USEG], [C, HALF], [1, WIN]],
                            ),
                        ).then_inc(my_sem, 16)
                        njobs += 1
            eng.wait_ge(my_sem, 16 * njobs)

        @block.sync
        def _(sync):
            emit_half(sync, 0, lmL, ltL, sS)

        @block.scalar
        def _(scalar):
            emit_half(scalar, 1, lmR, ltR, sA)

    return nc


_NC_CACHE = None


def make_in_maps(x):
    return [
        {"xp": np.pad(x[i], ((PAD, PAD), (PAD, PAD), (0, 0)))} for i in range(B)
    ]


def kernel(x) -> np.ndarray:
    global _NC_CACHE
    x = np.asarray(x, dtype=np.float32)
    assert x.shape == (B, H, W, C), x.shape
    if _NC_CACHE is None:
        _NC_CACHE = build_nc()
    in_maps = make_in_maps(x)
    res = run_bass_kernel_spmd(_NC_CACHE, in_maps, list(range(B)))
    outs = [res.results[i]["out"].reshape(H * W, K, K, C) for i in range(B)]
    return np.concatenate(outs, axis=0)


# revision 8
# speedup vs baseline: 1.4701x; 1.4701x over previous
"""NeighborSample Trainium2 kernel — SWDGE-exclusive stores + early start (v8).

Input  x:   (8, 64, 64, 192) f32
Output:     (8*64*64, 5, 5, 192) f32 — out[b*4096 + h*64 + w, i, j, c] =
            x[b, h+i-2, w+j-2, c] (zero-padded).

Trace-derived SDMA engine model (see v5/v7 notes):
  - an engine moves back-to-back 3840 B descriptors from ONE queue at
    ~149 ns each (25.8 GB/s); interleaving queues per-packet or using large
    descriptors halves that (~280 ns / ~14 GB/s);
  - HWDGE generates ~1 descriptor / 21 ns (too slow to feed 16 engines at
    the fast rate); SWDGE (gpsimd) generates much faster.

So: ALL 20 store DMAs (2 halves x 5 i x 2 h-segments, 1024 descriptors of
3840 B each) go on the single gpsimd SWDGE ring, which owns the engines
exclusively in steady state. Loads are host-padded xp rows on the two HWDGE
rings, split [32, 4, 32] rows so the g=0 stores can start once rows 0..35 of
both halves have landed (the g=1 rows stream in under the first stores).
"""

import sys

for _p in ("/opt/trn_rl_repo",):
    if _p not in sys.path:
        sys.path.insert(0, _p)

import numpy as np

import concourse.bass as bass
import concourse.mybir as mybir
from concourse.bass_utils import run_bass_kernel_spmd

B = 8
H = W = 64
C = 192
K = 5
PAD = 2
HALF = 32                # w positions per half
COLS = 36                # cols per half buffer
ROW = COLS * C           # 6912 elems per buf partition
PROWS = H + 2 * PAD      # 68 padded rows
PXROW = PROWS * C        # 13056 elems per padded-input row
WIN = K * C              # 960 (3840 B descriptor)
OUT_W = K * K * C        # 4800
OUT_H = W * OUT_W        # 307200
HSEG = 32                # h rows per store DMA


def build_nc() -> bass.Bass:
    nc = bass.Bass()
    xp = nc.declare_dram_parameter(
        "xp", [PROWS, PROWS, C], mybir.dt.float32, isOutput=False
    )
    out = nc.declare_dram_parameter(
        "out", [H, W, K, K, C], mybir.dt.float32, isOutput=True
    )

    with (
        nc.Block() as block,
        nc.semaphore("laL") as laL,
        nc.semaphore("la2L") as la2L,
        nc.semaphore("lbL") as lbL,
        nc.semaphore("laR") as laR,
        nc.semaphore("la2R") as la2R,
        nc.semaphore("lbR") as lbR,
        nc.semaphore("sP") as sP,
        nc.sbuf_tensor("bufL", [128, ROW], mybir.dt.float32) as bufL,
        nc.sbuf_tensor("bufR", [128, ROW], mybir.dt.float32) as bufR,
    ):
        bufs = [bufL, bufR]
        NJOBS = 20

        def emit_loads(eng, s, sems):
            col0 = s * HALF * C
            for r0, nr, sem in ((0, 32, sems[0]), (32, 4, sems[1]), (36, 32, sems[2])):
                eng.dma_start(
                    out=bass.AP(bufs[s], r0 * ROW, [[ROW, nr], [1, ROW]]),
                    in_=bass.AP(xp, r0 * PXROW + col0, [[PXROW, nr], [1, ROW]]),
                ).then_inc(sem, 16)

        @block.sync
        def _(sync):
            emit_loads(sync, 0, (laL, la2L, lbL))
            sync.wait_ge(sP, 16 * NJOBS)

        @block.scalar
        def _(scalar):
            emit_loads(scalar, 1, (laR, la2R, lbR))
            scalar.wait_ge(sP, 16 * NJOBS)

        @block.gpsimd
        def _(gpsimd):
            for sem in (laL, la2L, laR, la2R):
                gpsimd.wait_ge(sem, 16)
            for g in range(2):
                if g == 1:
                    gpsimd.wait_ge(lbL, 16)
                    gpsimd.wait_ge(lbR, 16)
                for i in range(K):
                    for s in range(2):
                        gpsimd.dma_start(
                            out=bass.AP(
                                out,
                                g * HSEG * OUT_H + s * HALF * OUT_W + i * WIN,
                                [[OUT_H, HSEG], [OUT_W, HALF], [1, WIN]],
                            ),
                            in_=bass.AP(
                                bufs[s],
                                (i + g * HSEG) * ROW,
                                [[ROW, HSEG], [C, HALF], [1, WIN]],
                            ),
                            single_packet=True,
                        ).then_inc(sP, 16)
            gpsimd.wait_ge(sP, 16 * NJOBS)

    return nc


_NC_CACHE = None


def make_in_maps(x):
    return [
        {"xp": np.pad(x[i], ((PAD, PAD), (PAD, PAD), (0, 0)))} for i in range(B)
    ]


def kernel(x) -> np.ndarray:
    global _NC_CACHE
    x = np.asarray(x, dtype=np.float32)
    assert x.shape == (B, H, W, C), x.shape
    if _NC_CACHE is None:
        _NC_CACHE = build_nc()
    in_maps = make_in_maps(x)
    res = run_bass_kernel_spmd(_NC_CACHE, in_maps, list(range(B)))
    outs = [res.results[i]["out"].reshape(H * W, K, K, C) for i in range(B)]
    return np.concatenate(outs, axis=0)


# revision 9
# speedup vs baseline: 2.0036x; 1.3629x over previous
"""NeighborSample Trainium2 kernel — cleaned two-ring v1 dataflow (v10).

Input  x:   (8, 64, 64, 192) f32
Output:     (8*64*64, 5, 5, 192) f32 — out[b*4096 + h*64 + w, i, j, c] =
            x[b, h+i-2, w+j-2, c] (zero-padded).

v1's dataflow (3840 B sliding-window descriptors, left half on partitions
0..63 under the SP HWDGE ring, right half on 64..127 under the ACT ring —
fully partition-disjoint queues) measured best so far. Its losses were:
serial DVE memsets before the loads, zbuf zero-row stores CONFLICTING with
bufL's partitions, and 2-engine fan-out hazards. v10 removes all three:
  - x is host-padded to xp [68, 68, 192]; the two interior loads read the
    w-padded columns directly (no memsets, 27648 B descriptors, 16-engine
    fan-out);
  - zero output rows (patch rows falling outside the image) are written
    from xp's row 0 — which is all zeros — in DRAM, so the zero stores
    touch NO SBUF partition at all;
  - store segments are [48, cnt-48] as in v1 (fan-out 16 + 14/15/16).
"""

import sys

for _p in ("/opt/trn_rl_repo",):
    if _p not in sys.path:
        sys.path.insert(0, _p)

import numpy as np

import concourse.bass as bass
import concourse.mybir as mybir
from concourse.bass_utils import run_bass_kernel_spmd

B = 8
H = W = 64
C = 192
K = 5
PAD = 2
HALF = 32
COLS = 36
ROW = COLS * C           # 6912
PROWS = H + 2 * PAD      # 68
PXROW = PROWS * C        # 13056
WIN = K * C              # 960
OUT_W = K * K * C        # 4800
OUT_H = W * OUT_W        # 307200


def build_nc() -> bass.Bass:
    nc = bass.Bass()
    xp = nc.declare_dram_parameter(
        "xp", [PROWS, PROWS, C], mybir.dt.float32, isOutput=False
    )
    out = nc.declare_dram_parameter(
        "out", [H, W, K, K, C], mybir.dt.float32, isOutput=True
    )

    with (
        nc.Block() as block,
        nc.semaphore("lL") as lL,
        nc.semaphore("lR") as lR,
        nc.semaphore("sS") as sS,
        nc.semaphore("sA") as sA,
        nc.sbuf_tensor("buf", [128, ROW], mybir.dt.float32) as buf,
    ):

        def emit_half(eng, s, lsem, my_sem):
            p0 = s * 64
            col0 = s * HALF * C
            # interior load: partitions p0..p0+63 <- xp rows 2..65 half-cols
            eng.dma_start(
                out=bass.AP(buf, p0 * ROW, [[ROW, 64], [1, ROW]]),
                in_=bass.AP(xp, PAD * PXROW + col0, [[PXROW, 64], [1, ROW]]),
            ).then_inc(lsem, 16)
            eng.wait_ge(lsem, 16)
            n = 0
            # main stores: valid h rows, chopped [48, cnt-48] for fan-out
            for i in range(K):
                h0 = max(0, PAD - i)
                h1 = min(H, H + PAD - i)
                cnt = h1 - h0
                segs = [cnt] if cnt == 48 or cnt % 16 == 0 else [48, cnt - 48]
                st = h0
                for seg in segs:
                    eng.dma_start(
                        out=bass.AP(
                            out,
                            st * OUT_H + s * HALF * OUT_W + i * WIN,
                            [[OUT_H, seg], [OUT_W, HALF], [1, WIN]],
                        ),
                        in_=bass.AP(
                            buf,
                            (p0 + st + i - PAD) * ROW,
                            [[ROW, seg], [C, HALF], [1, WIN]],
                        ),
                    ).then_inc(my_sem, 16)
                    st += seg
                    n += 1
            # zero stores: patch rows outside the image, sourced from xp row 0
            # (all zeros, in DRAM -> no SBUF partition traffic)
            for i, hz, zcnt in ((0, 0, 2), (1, 0, 1), (3, 63, 1), (4, 62, 2)):
                eng.dma_start(
                    out=bass.AP(
                        out,
                        hz * OUT_H + s * HALF * OUT_W + i * WIN,
                        [[OUT_H, zcnt], [OUT_W, HALF], [1, WIN]],
                    ),
                    in_=bass.AP(xp, 0, [[0, zcnt], [0, HALF], [1, WIN]]),
                ).then_inc(my_sem, 16)
                n += 1
            eng.wait_ge(my_sem, 16 * n)

        @block.sync
        def _(sync):
            emit_half(sync, 0, lL, sS)

        @block.scalar
        def _(scalar):
            emit_half(scalar, 1, lR, sA)

    return nc


_NC_CACHE = None


def make_in_maps(x):
    return [
        {"xp": np.pad(x[i], ((PAD, PAD), (PAD, PAD), (0, 0)))} for i in range(B)
    ]


def kernel(x) -> np.ndarray:
    global _NC_CACHE
    x = np.asarray(x, dtype=np.float32)
    assert x.shape == (B, H, W, C), x.shape
    if _NC_CACHE is None:
        _NC_CACHE = build_nc()
    in_maps = make_in_maps(x)
    res = run_bass_kernel_spmd(_NC_CACHE, in_maps, list(range(B)))
    outs = [res.results[i]["out"].reshape(H * W, K, K, C) for i in range(B)]
    return np.concatenate(outs, axis=0)
